# revision 1
# baseline (speedup 1.0000x reference)
"""Trainium2 Bass kernel for DecisionTreeModule forward.

Computes, for x [B, 256]: a 12-level complete-binary-tree traversal
(per-sample feature compares) followed by softmax(leaf_probabilities[leaf]).

Strategy (8 NeuronCores, pure data parallel over the batch):
  - Each core gets a padded shard of rows. Samples live one-per-partition in
    groups of G tiles ([128, G, 256] SBUF tiles).
  - Levels 0-6: node (feat, thr) looked up from per-level replicated SBUF
    tables by one-hot mask + segmented reduce (exact f32 select).
  - Levels 7-11: one indirect-DMA fetch per tile pulls a 62-float record
    (the (feat, thr) pairs of the whole 5-level subtree under the sample's
    level-7 node); within-record selects are narrow (<=16 wide).
  - The x-value select per level is a 256-wide one-hot mask + segmented
    reduce against the resident x tile (exact: 1.0*x + zeros).
  - Output: softmax table [4096, 100] built once on device from
    leaf_probabilities; per-tile indirect DMA gathers out[p] = smx[leaf[p]].

All compares are exact f32, so leaf indices match the reference exactly;
only the softmax arithmetic carries rounding error.
"""
import sys
sys.path.insert(0, "/opt/trn_rl_repo")

import numpy as np
import concourse.bacc as bacc
import concourse.bass as bass
import concourse.mybir as mybir
import concourse.tile as tile
from concourse.bass_utils import run_bass_kernel_spmd

P = 128
INPUT_DIM = 256
N_CLASSES = 100
MAX_DEPTH = 12
N_NODES = 2 ** MAX_DEPTH - 1     # 4095
N_LEAVES = 2 ** MAX_DEPTH        # 4096
NCORES = 8
REC_W = 62                       # 31 (feat, thr) pairs: levels 7..11 subtree

F32 = mybir.dt.float32
I32 = mybir.dt.int32
Alu = mybir.AluOpType


def _build_program(G: int, NG: int, f0: float, t0: float, repeat: int = 1):
    """Build the per-core Bass program. S = 128*G*NG samples."""
    S = P * G * NG
    nc = bacc.Bacc("TRN2", target_bir_lowering=False, debug=False)

    x = nc.dram_tensor("x", [S, INPUT_DIM], F32, kind="ExternalInput")
    lp = nc.dram_tensor("lp", [N_LEAVES, N_CLASSES], F32, kind="ExternalInput")
    iota = nc.dram_tensor("iota", [P, INPUT_DIM], F32, kind="ExternalInput")
    iotab = nc.dram_tensor("iotab", [P, INPUT_DIM], mybir.dt.bfloat16, kind="ExternalInput")
    # per-level (feat, thr) tables for levels 1..6, replicated per partition
    ftlev = {
        d: nc.dram_tensor(f"ft{d}", [P, 2, 2 ** d], F32, kind="ExternalInput")
        for d in range(1, 7)
    }
    rectab = nc.dram_tensor("rectab", [P, REC_W], F32, kind="ExternalInput")
    out = nc.dram_tensor("out", [S, N_CLASSES], F32, kind="ExternalOutput")
    smx = nc.dram_tensor("smx", [N_LEAVES, N_CLASSES], F32, kind="Internal")

    xg_all = x[:, :].rearrange("(g t p) f -> p g t f", p=P, t=G)
    og_all = out[:, :].rearrange("(g t p) c -> p g t c", p=P, t=G)
    lp_r = lp[:, :].rearrange("(p c) k -> p c k", p=P)
    smx_r = smx[:, :].rearrange("(p c) k -> p c k", p=P)

    with tile.TileContext(nc) as tc:
        with tc.tile_pool(name="cns", bufs=1) as cpool, \
             tc.tile_pool(name="xg", bufs=2) as xpool, \
             tc.tile_pool(name="mask", bufs=1) as mpool, \
             tc.tile_pool(name="xm", bufs=2) as xmpool, \
             tc.tile_pool(name="xp", bufs=2) as xppool, \
             tc.tile_pool(name="prod", bufs=1) as ppool, \
             tc.tile_pool(name="sml", bufs=3) as spool, \
             tc.tile_pool(name="rec", bufs=2) as rpool, \
             tc.tile_pool(name="orow", bufs=2) as opool:

            # ---- constants into SBUF ----
            t_iota = cpool.tile([P, 1, INPUT_DIM], F32)
            nc.sync.dma_start(t_iota[:], iota[:, :].rearrange("p (o f) -> p o f", o=1))
            t_iotab = cpool.tile([P, 1, INPUT_DIM], mybir.dt.bfloat16)
            nc.sync.dma_start(t_iotab[:], iotab[:, :].rearrange("p (o f) -> p o f", o=1))
            t_ft = {}
            for d in range(1, 7):
                t_ftd = cpool.tile([P, 1, 2, 2 ** d], F32, tag=f"ft{d}")
                nc.sync.dma_start(t_ftd[:], ftlev[d][:, :, :].rearrange("(p o) c w -> p o c w", o=1))
                t_ft[d] = t_ftd

            # ---- Part 1: softmax table smx = softmax(lp, axis=1) ----
            with tc.tile_pool(name="p1", bufs=1) as p1pool:
                t_lp = p1pool.tile([P, 32, N_CLASSES], F32)
                nc.sync.dma_start(t_lp[:], lp_r[:, :, :])
                t_exp = p1pool.tile([P, 32, N_CLASSES], F32)
                nc.scalar.activation(out=t_exp[:], in_=t_lp[:],
                                     func=mybir.ActivationFunctionType.Exp)
                t_sum = p1pool.tile([P, 32, 1], F32)
                nc.vector.tensor_reduce(t_sum[:], t_exp[:], mybir.AxisListType.X, Alu.add)
                t_rcp = p1pool.tile([P, 32, 1], F32)
                nc.vector.reciprocal(t_rcp[:], t_sum[:])
                nc.vector.tensor_tensor(
                    out=t_exp[:], in0=t_exp[:],
                    in1=t_rcp[:, :, :].to_broadcast([P, 32, N_CLASSES]),
                    op=Alu.mult)
                nc.sync.dma_start(smx_r[:, :, :], t_exp[:])

            # ---- Part 2: traversal per group ----
            rep_ctx = tc.For_i(0, repeat, 1) if repeat > 1 else None
            if rep_ctx is not None:
                rep_ctx.__enter__()
            for g in range(NG):
                t_x = xpool.tile([P, G, INPUT_DIM], F32, tag="x")
                nc.sync.dma_start(t_x[:], xg_all[:, g])

                node = None    # [P, G] f32, level-local node idx (levels 0-6)
                lnode = None   # [P, G] f32, subtree-local (levels 7-11)
                node7 = None
                t_rec = None

                for d in range(MAX_DEPTH):
                    # --- (feat, thr) for this level -> ft [P, G, 2] (or imm) ---
                    ft = None
                    if d == 0:
                        pass  # immediates f0, t0
                    elif d <= 6:
                        W = 2 ** d
                        t_nm = mpool.tile([P, G, 64], F32, tag="nmask")
                        nm = t_nm[:, :, :W]
                        nc.vector.tensor_tensor(
                            out=nm, in0=t_iota[:, :, :W].to_broadcast([P, G, W]),
                            in1=node[:, :, :].to_broadcast([P, G, W]),
                            op=Alu.is_equal)
                        t_pr = ppool.tile([P, G, 2, 64], F32, tag="nprod")
                        pr = t_pr[:, :, :, :W]
                        nc.vector.tensor_tensor(
                            out=pr,
                            in0=t_nm[:, :, :W].rearrange("p g (o w) -> p g o w", o=1).to_broadcast([P, G, 2, W]),
                            in1=t_ft[d][:, :, :, :W].to_broadcast([P, G, 2, W]),
                            op=Alu.mult)
                        ft = spool.tile([P, G, 2], F32, tag="ft")
                        nc.vector.tensor_reduce(ft[:], pr, mybir.AxisListType.X, Alu.add)
                    elif d == 7:
                        ft = t_rec[:, :, 0:2]
                    else:
                        j = d - 7
                        W = 2 ** j
                        base = 2 * (W - 1)
                        t_lm = mpool.tile([P, G, 16], F32, tag="lmask")
                        lm = t_lm[:, :, :W]
                        nc.vector.tensor_tensor(
                            out=lm, in0=t_iota[:, :, :W].to_broadcast([P, G, W]),
                            in1=lnode[:, :, :].to_broadcast([P, G, W]),
                            op=Alu.is_equal)
                        # record view [P, G, 2, W]: elem (c, l) at base + 2l + c
                        rv = t_rec[:, :, base:base + 2 * W].rearrange(
                            "p g (l c) -> p g c l", c=2)
                        t_pr = ppool.tile([P, G, 2, 16], F32, tag="lprod")
                        pr = t_pr[:, :, :, :W]
                        nc.vector.tensor_tensor(
                            out=pr,
                            in0=t_lm[:, :, :W].rearrange("p g (o w) -> p g o w", o=1).to_broadcast([P, G, 2, W]),
                            in1=rv, op=Alu.mult)
                        ft = spool.tile([P, G, 2], F32, tag="ft")
                        nc.vector.tensor_reduce(ft[:], pr, mybir.AxisListType.X, Alu.add)

                    # --- x-value select: val = x[s, feat] ---
                    t_xp = xppool.tile([P, G, INPUT_DIM], F32, tag="xprod")
                    if d == 0:
                        # fused: xprod = (iota == f0) * x
                        nc.vector.scalar_tensor_tensor(
                            out=t_xp[:],
                            in0=t_iota[:, :, :].to_broadcast([P, G, INPUT_DIM]),
                            scalar=f0, in1=t_x[:],
                            op0=Alu.is_equal, op1=Alu.mult)
                    else:
                        ftb = spool.tile([P, G, 1], mybir.dt.bfloat16, tag="ftb")
                        nc.vector.tensor_copy(out=ftb[:], in_=ft[:, :, 0:1])
                        t_xm = xmpool.tile([P, G, INPUT_DIM], mybir.dt.bfloat16,
                                           tag="xmask")
                        nc.vector.tensor_tensor(
                            out=t_xm[:],
                            in0=t_iotab[:, :, :].to_broadcast([P, G, INPUT_DIM]),
                            in1=ftb[:, :, :].to_broadcast([P, G, INPUT_DIM]),
                            op=Alu.is_equal)
                        nc.vector.tensor_tensor(out=t_xp[:], in0=t_xm[:], in1=t_x[:],
                                                op=Alu.mult)
                    val = spool.tile([P, G, 1], F32, tag="val")
                    nc.vector.tensor_reduce(val[:], t_xp[:], mybir.AxisListType.X,
                                            Alu.add)

                    # --- bit + node update ---
                    bit = spool.tile([P, G, 1], F32, tag="bit")
                    if d == 0:
                        nc.vector.tensor_scalar(
                            out=bit[:], in0=val[:], scalar1=t0, scalar2=None,
                            op0=Alu.is_gt)
                    else:
                        nc.vector.tensor_tensor(out=bit[:], in0=val[:],
                                                in1=ft[:, :, 1:2], op=Alu.is_gt)

                    if d == 0:
                        node = bit
                    elif d < 7:
                        nn = spool.tile([P, G, 1], F32, tag="node")
                        nc.vector.scalar_tensor_tensor(
                            out=nn[:], in0=node[:], scalar=2.0, in1=bit[:],
                            op0=Alu.mult, op1=Alu.add)
                        node = nn
                    elif d == 7:
                        lnode = bit
                    else:
                        ln = spool.tile([P, G, 1], F32, tag="lnode")
                        nc.vector.scalar_tensor_tensor(
                            out=ln[:], in0=lnode[:], scalar=2.0, in1=bit[:],
                            op0=Alu.mult, op1=Alu.add)
                        lnode = ln

                    if d == 6:
                        node7 = node
                        reci = spool.tile([P, G], I32, tag="reci")
                        nc.vector.tensor_copy(out=reci[:], in_=node[:])
                        t_rec = rpool.tile([P, G, REC_W], F32, tag="rec")
                        for t in range(G):
                            nc.gpsimd.indirect_dma_start(
                                out=t_rec[:, t, :], out_offset=None,
                                in_=rectab[:, :],
                                in_offset=bass.IndirectOffsetOnAxis(
                                    ap=reci[:, t:t + 1], axis=0))

                # leaf = node7 * 32 + lnode
                leaf = spool.tile([P, G, 1], F32, tag="leaf")
                nc.vector.scalar_tensor_tensor(
                    out=leaf[:], in0=node7[:], scalar=32.0, in1=lnode[:],
                    op0=Alu.mult, op1=Alu.add)
                leafi = spool.tile([P, G], I32, tag="leafi")
                nc.vector.tensor_copy(out=leafi[:], in_=leaf[:])

                t_or = opool.tile([P, G, N_CLASSES], F32, tag="orow")
                for t in range(G):
                    nc.gpsimd.indirect_dma_start(
                        out=t_or[:, t, :], out_offset=None, in_=smx[:, :],
                        in_offset=bass.IndirectOffsetOnAxis(
                            ap=leafi[:, t:t + 1], axis=0))
                nc.sync.dma_start(og_all[:, g], t_or[:])

            if rep_ctx is not None:
                rep_ctx.__exit__(None, None, None)

    nc.compile()
    return nc


def _host_tables(split_features, split_thresholds):
    feat = np.clip(np.floor(split_features), 0, INPUT_DIM - 1).astype(np.int64)
    thr = split_thresholds.astype(np.float32)
    featf = feat.astype(np.float32)

    iota = np.broadcast_to(np.arange(INPUT_DIM, dtype=np.float32),
                           (P, INPUT_DIM)).copy()
    ftlev = {}
    for d in range(1, 7):
        W = 2 ** d
        lo = W - 1
        tab = np.empty((2, W), np.float32)
        tab[0] = featf[lo:lo + W]
        tab[1] = thr[lo:lo + W]
        ftlev[d] = np.broadcast_to(tab, (P, 2, W)).copy()

    rec = np.empty((P, REC_W), np.float32)
    for l7 in range(P):
        for j in range(5):
            W = 2 ** j
            lvl_base = 2 ** (7 + j) - 1
            for l in range(W):
                n = lvl_base + l7 * W + l
                off = 2 * (W - 1 + l)
                rec[l7, off] = featf[n]
                rec[l7, off + 1] = thr[n]
    f0 = float(featf[0])
    t0 = float(thr[0])
    return iota, ftlev, rec, f0, t0


def _to_bf16(a):
    import ml_dtypes
    return a.astype(ml_dtypes.bfloat16)


_PROG_CACHE = {}


def kernel(x, split_features, split_thresholds, leaf_probabilities):
    x = np.asarray(x, dtype=np.float32)
    split_features = np.asarray(split_features, dtype=np.float32)
    split_thresholds = np.asarray(split_thresholds, dtype=np.float32)
    leaf_probabilities = np.asarray(leaf_probabilities, dtype=np.float32)

    B = x.shape[0]
    G = 24                                  # tiles per group
    per_core_min = (B + NCORES - 1) // NCORES
    tiles_pc = (per_core_min + P - 1) // P  # tiles needed per core
    NG = (tiles_pc + G - 1) // G            # groups per core
    S = P * G * NG                          # padded samples per core

    iota, ftlev, rec, f0, t0 = _host_tables(split_features, split_thresholds)

    key = (G, NG, f0, t0)
    nc = _PROG_CACHE.get(key)
    if nc is None:
        nc = _build_program(G, NG, f0, t0)
        _PROG_CACHE[key] = nc

    in_maps = []
    for c in range(NCORES):
        lo = c * S
        hi = min(lo + S, B)
        shard = np.empty((S, INPUT_DIM), np.float32)
        if hi > lo:
            shard[:hi - lo] = x[lo:hi]
            if hi - lo < S:
                shard[hi - lo:] = x[0]
        else:
            shard[:] = x[0]
        m = {"x": shard, "lp": leaf_probabilities, "iota": iota,
             "iotab": _to_bf16(iota),
             "rectab": rec}
        for d in range(1, 7):
            m[f"ft{d}"] = ftlev[d]
        in_maps.append(m)

    res = run_bass_kernel_spmd(nc, in_maps, core_ids=list(range(NCORES)))

    outs = []
    for c in range(NCORES):
        lo = c * S
        hi = min(lo + S, B)
        if hi > lo:
            outs.append(res.results[c]["out"][:hi - lo])
    return np.concatenate(outs, axis=0)

